# revision 1
# baseline (speedup 1.0000x reference)
"""ContrastiveSparseRepresentation TRN2 kernel.

out = normalize(topk_mask(layernorm(x @ W + b) * gamma + beta, k=64))

Math used (valid for b=0, beta=0, gamma=const>0, per the problem spec):
  p = (h - mu) * rsqrt(var + eps) * g;  topk by |p| == topk by |h - mu|;
  normalize(mask * p) == mask * (h - mu) / ||mask * (h - mu)||  (g, rsqrt cancel)

Sharding: data-parallel over the 32768-row batch across 8 NeuronCores.
Per core: 4096 rows = 32 tiles of 128 rows (partition dim).

Per tile:
  PE   : h[128,4096] = x_tile @ W  (fp32, 8 PSUM banks x 6 K-chunks)
  ACT  : drain PSUM->SBUF with accum_out (row sums -> mu); a = |h - mu|
  DVE  : 64x max8 over segments of 64 -> cand[128,512]
         8x (max8 + match_replace) rounds -> top-64 values; t = 64th value
         mask = (a >= t)  (in place on a); out = hm * mask (in place on a)
  GPS  : hm = (h - mu) * rsqrt(sum(top64^2))  (in place on h)
"""

import numpy as np
from contextlib import ExitStack

import concourse.bass as bass
import concourse.tile as tile
from concourse import bacc, mybir
from concourse import bass_utils
from concourse.alu_op_type import AluOpType

F32 = mybir.dt.float32
AF = mybir.ActivationFunctionType
AX = mybir.AxisListType

B, D_IN, D_OUT = 32768, 768, 4096
N_CORES = 8
R = B // N_CORES            # rows per core
P = 128                     # rows per tile (partition dim)
N_TILES = R // P            # 32
KC = D_IN // P              # 6 contraction chunks
NBANK = D_OUT // 512        # 8 psum banks
SEG = 64
NSEG = D_OUT // SEG         # 64 segments
K = 64                      # top-k
NEG = -1e30

_CACHE = {}
MATMUL_MODE = "f16x3"  # "f32" (exact, 4 cyc/row) | "f16x3" (hi/lo split, 25% faster PE)
F16 = mybir.dt.float16


def _build(n_tiles=N_TILES, stage=5, mode=None):
    mode = mode or MATMUL_MODE
    nc = bacc.Bacc("TRN2", target_bir_lowering=False, debug=False,
                   num_devices=N_CORES, enable_asserts=False)
    xT = nc.dram_tensor("xT", [D_IN, R], F32, kind="ExternalInput").ap()
    W = nc.dram_tensor("W", [D_IN, D_OUT], F32, kind="ExternalInput").ap()
    out = nc.dram_tensor("out", [R, D_OUT], F32, kind="ExternalOutput").ap()

    with tile.TileContext(nc) as tc, ExitStack() as ctx:
        wp = ctx.enter_context(tc.tile_pool(name="w", bufs=1))
        xp = ctx.enter_context(tc.tile_pool(name="x", bufs=2))
        hp = ctx.enter_context(tc.tile_pool(name="h", bufs=2))
        ap_ = ctx.enter_context(tc.tile_pool(name="a", bufs=2))
        cp = ctx.enter_context(tc.tile_pool(name="c", bufs=2))
        sp = ctx.enter_context(tc.tile_pool(name="s", bufs=2))
        pp = ctx.enter_context(tc.tile_pool(name="ps", bufs=8, space="PSUM"))

        if mode == "f32":
            w_t = wp.tile([P, KC * D_OUT], F32, tag="w")
            for k in range(KC):
                nc.sync.dma_start(w_t[:, k * D_OUT:(k + 1) * D_OUT],
                                  W[k * P:(k + 1) * P, :])
        else:  # f16x3: resident hi/lo fp16 halves of W
            w16h = wp.tile([P, KC * D_OUT], F16, tag="wh")
            w16l = wp.tile([P, KC * D_OUT], F16, tag="wl")
            for k in range(KC):
                wtmp = hp.tile([P, D_OUT], F32, tag="h")
                nc.sync.dma_start(wtmp[:], W[k * P:(k + 1) * P, :])
                sl = slice(k * D_OUT, (k + 1) * D_OUT)
                nc.vector.tensor_copy(w16h[:, sl], wtmp[:])
                nc.vector.tensor_tensor(out=w16l[:, sl], in0=wtmp[:],
                                        in1=w16h[:, sl],
                                        op=AluOpType.subtract)

        for it in range(n_tiles):
            # x tile: [128 k-part, 6 chunks * 128 rows]
            x_t = xp.tile([P, KC * P], F32, tag="x")
            for k in range(KC):
                nc.sync.dma_start(x_t[:, k * P:(k + 1) * P],
                                  xT[k * P:(k + 1) * P, it * P:(it + 1) * P])

            if mode == "f16x3":
                xh = xp.tile([P, KC * P], F16, tag="xh")
                xl = xp.tile([P, KC * P], F16, tag="xl")
                nc.scalar.copy(xh[:], x_t[:])
                nc.vector.tensor_tensor(out=xl[:], in0=x_t[:], in1=xh[:],
                                        op=AluOpType.subtract)

            hs = hp.tile([P, D_OUT], F32, tag="h")
            sparts = sp.tile([P, NBANK], F32, tag="sparts")
            for b in range(NBANK):
                ps = pp.tile([P, 512], F32, tag="ps")
                if mode == "f32":
                    for k in range(KC):
                        nc.tensor.matmul(
                            ps[:],
                            x_t[:, k * P:(k + 1) * P],
                            w_t[:, k * D_OUT + b * 512: k * D_OUT + (b + 1) * 512],
                            start=(k == 0), stop=(k == KC - 1))
                else:
                    n_mm = 3 * KC
                    i = 0
                    for k in range(KC):
                        xs = slice(k * P, (k + 1) * P)
                        ws = slice(k * D_OUT + b * 512, k * D_OUT + (b + 1) * 512)
                        for lhs, rhs in ((xh, w16h), (xh, w16l), (xl, w16h)):
                            nc.tensor.matmul(ps[:], lhs[:, xs], rhs[:, ws],
                                             start=(i == 0), stop=(i == n_mm - 1))
                            i += 1
                nc.scalar.activation(hs[:, b * 512:(b + 1) * 512], ps[:],
                                     AF.Copy, accum_out=sparts[:, b:b + 1])

            if stage <= 1:
                nc.sync.dma_start(out[it * P:(it + 1) * P, :], hs[:])
                continue

            ssum = sp.tile([P, 1], F32, tag="ssum")
            nc.vector.reduce_sum(ssum[:], sparts[:], axis=AX.X)
            negmu = sp.tile([P, 1], F32, tag="negmu")
            nc.vector.tensor_scalar(out=negmu[:], in0=ssum[:],
                                    scalar1=-1.0 / D_OUT, scalar2=None,
                                    op0=AluOpType.mult)

            # a = |h - mu|
            a_t = ap_.tile([P, D_OUT], F32, tag="a")
            nc.scalar.activation(a_t[:], hs[:], AF.Abs, bias=negmu[:], scale=1.0)

            if stage <= 2:
                nc.sync.dma_start(out[it * P:(it + 1) * P, :], a_t[:])
                continue

            # L1: per-segment top-8 candidates
            cand = cp.tile([P, NSEG * 8], F32, tag="cand")
            for s in range(NSEG):
                nc.vector.max(cand[:, s * 8:(s + 1) * 8],
                              a_t[:, s * SEG:(s + 1) * SEG])

            if stage <= 3:
                nc.sync.dma_start(out[it * P:(it + 1) * P, 0:NSEG * 8], cand[:])
                continue

            # L2: 8 rounds of max8 + match_replace -> top-64 values
            vals = cp.tile([P, K], F32, tag="vals")
            cur = cand
            for r in range(K // 8):
                nc.vector.max(vals[:, r * 8:(r + 1) * 8], cur[:])
                if r < K // 8 - 1:
                    nxt = cp.tile([P, NSEG * 8], F32, tag=f"mr{r % 2}")
                    nc.vector.match_replace(nxt[:], vals[:, r * 8:(r + 1) * 8],
                                            cur[:], NEG)
                    cur = nxt

            if stage <= 4:
                nc.sync.dma_start(out[it * P:(it + 1) * P, 0:K], vals[:])
                continue

            # norm scale: shat = sqrt(1 / sum(vals^2))
            sq = sp.tile([P, K], F32, tag="sq")
            ss = sp.tile([P, 1], F32, tag="ss")
            nc.scalar.activation(sq[:], vals[:], AF.Square, accum_out=ss[:])
            rr = sp.tile([P, 1], F32, tag="rr")
            nc.vector.reciprocal(rr[:], ss[:])
            shat = sp.tile([P, 1], F32, tag="shat")
            nc.scalar.activation(shat[:], rr[:], AF.Sqrt)

            # mask = (a >= t) in place; hm = h - mu in place
            nc.vector.tensor_scalar(out=a_t[:], in0=a_t[:],
                                    scalar1=vals[:, K - 1:K], scalar2=None,
                                    op0=AluOpType.is_ge)
            nc.scalar.activation(hs[:], hs[:], AF.Identity, bias=negmu[:],
                                 scale=1.0)
            # masked = hm * mask (into a's slot); scale by shat via ACT copy
            nc.vector.tensor_tensor(out=a_t[:], in0=hs[:], in1=a_t[:],
                                    op=AluOpType.mult)
            nc.scalar.activation(a_t[:], a_t[:], AF.Copy, scale=shat[:])
            nc.sync.dma_start(out[it * P:(it + 1) * P, :], a_t[:])

    nc.compile()
    return nc


def _get_nc():
    if "nc" not in _CACHE:
        _CACHE["nc"] = _build()
    return _CACHE["nc"]


def _numpy_fallback(x, W, b, gamma, beta):
    h = x.astype(np.float32) @ W.astype(np.float32) + b
    mu = h.mean(-1, keepdims=True)
    var = np.square(h - mu).mean(-1, keepdims=True)
    p = (h - mu) / np.sqrt(var + 1e-5) * gamma + beta
    idx = np.argsort(-np.abs(p), axis=-1, kind="stable")[:, :K]
    sparse = np.zeros_like(p)
    np.put_along_axis(sparse, idx, np.take_along_axis(p, idx, -1), -1)
    nrm = np.linalg.norm(sparse, axis=-1, keepdims=True)
    return sparse / np.maximum(nrm, 1e-12)


def kernel(**inputs):
    x = np.ascontiguousarray(np.asarray(inputs["x"], dtype=np.float32))
    W = np.ascontiguousarray(np.asarray(inputs["W"], dtype=np.float32))
    b = np.asarray(inputs["b"], dtype=np.float32)
    gamma = np.asarray(inputs["gamma"], dtype=np.float32)
    beta = np.asarray(inputs["beta"], dtype=np.float32)

    # kernel math relies on b == 0, beta == 0, gamma == const > 0 (per spec)
    if (np.any(b != 0) or np.any(beta != 0)
            or np.any(gamma != gamma[0]) or gamma[0] <= 0):
        return _numpy_fallback(x, W, b, gamma, beta)

    xT = np.ascontiguousarray(x.T)  # [768, 32768]
    in_maps = [
        {"xT": np.ascontiguousarray(xT[:, c * R:(c + 1) * R]), "W": W}
        for c in range(N_CORES)
    ]
    nc = _get_nc()
    import os
    trace = os.environ.get("KERNEL_TRACE") == "1"
    try:
        res = bass_utils.run_bass_kernel_spmd(
            nc, in_maps, core_ids=list(range(N_CORES)), trace=trace,
            trace_cores=[0] if trace else None)
    except Exception:
        if not trace:
            raise
        res = bass_utils.run_bass_kernel_spmd(
            nc, in_maps, core_ids=list(range(N_CORES)))
    _CACHE["last_res"] = res
    return np.concatenate([res.results[c]["out"] for c in range(N_CORES)],
                          axis=0)



# revision 3
# speedup vs baseline: 4.8532x; 4.8532x over previous
"""ContrastiveSparseRepresentation TRN2 kernel.

out = normalize(topk_mask(layernorm(x @ W + b) * gamma + beta, k=64))

Math used (valid for b=0, beta=0, gamma=const>0, per the problem spec):
  p = (h - mu) * rsqrt(var + eps) * g;  topk by |p| == topk by |h - mu|;
  normalize(mask * p) == mask * (h - mu) / ||mask * (h - mu)||  (g, rsqrt cancel)

Wall-clock here is dominated by host<->device transfer over the axon tunnel
(~100 MB/s), so the kernel minimizes bytes moved:
  - x is uploaded as fp16 (50 MB) plus an fp8-e4m3 low-order correction
    (x - fp16(x)) * 2048 (25 MB).  On device the correction is scaled back
    and a 3-term fp16 matmul (xh*Wh + xh*Wl + xl*Wh) reproduces fp32-level
    accuracy (h err ~1e-5), which keeps top-64 support flips to ~25 rows.
  - W is uploaded pre-split into fp16 hi/lo halves, sharded by column
    across the 8 cores (1.6 MB/core), and assembled on-device with an
    HBM-to-HBM AllGather: 12.6 MB total instead of 100 MB replicated.
  - The device returns only the top-64 per row: packed values [R,64] f32
    (|h-mu| with sign stuffed into mantissa bit 0) + column indices [R,64]
    u16 recovered with DVE max_index.  Download is 12 MB instead of 512 MB.
    The host decodes, normalizes, scatters to dense, and exactly recomputes
    the rare rows where a duplicated |h-mu| value made max_index return the
    same column twice.

Sharding: data-parallel over the 32768-row batch across 8 NeuronCores.
Per core: 4096 rows = 32 tiles of 128 rows (partition dim).
"""

import numpy as np
from contextlib import ExitStack

import ml_dtypes
import concourse.bass as bass
import concourse.tile as tile
from concourse import bacc, masks, mybir
from concourse import bass_utils
from concourse.alu_op_type import AluOpType

F32 = mybir.dt.float32
F16 = mybir.dt.float16
F8 = mybir.dt.float8e4
I32 = mybir.dt.int32
U16 = mybir.dt.uint16
AF = mybir.ActivationFunctionType
AX = mybir.AxisListType

B, D_IN, D_OUT = 32768, 768, 4096
N_CORES = 8
R = B // N_CORES            # rows per core
P = 128                     # rows per tile (partition dim)
N_TILES = R // P            # 32
KC = D_IN // P              # 6 contraction chunks
NBANK = D_OUT // 512        # 8 psum banks
WSL = D_OUT // N_CORES      # 512: W columns owned per core
SEG = 64
NSEG = D_OUT // SEG         # 64 segments
K = 64                      # top-k
NEG = -1e30
XL_SCALE = 2048.0           # 2**11: fp8 correction prescale

_CACHE = {}


def _build(n_tiles=N_TILES, stage=5):
    nc = bacc.Bacc("TRN2", target_bir_lowering=False, debug=False,
                   num_devices=N_CORES, enable_asserts=False)
    x16 = nc.dram_tensor("x16", [R, D_IN], F16, kind="ExternalInput").ap()
    xl8 = nc.dram_tensor("xl8", [R, D_IN], F8, kind="ExternalInput").ap()
    # [Wh ; Wl] fp16, this core's 512-column slice
    whl = nc.dram_tensor("whl", [2 * D_IN, WSL], F16,
                         kind="ExternalInput").ap()
    wstg = nc.dram_tensor("wstg", [2 * D_IN, WSL], F16, kind="Internal").ap()
    wgth = nc.dram_tensor("wgth", [N_CORES * 2 * D_IN, WSL], F16,
                          kind="Internal").ap()
    vout = nc.dram_tensor("vout", [R, K], F32, kind="ExternalOutput").ap()
    iout = nc.dram_tensor("iout", [R, K], U16, kind="ExternalOutput").ap()
    dbg = None
    if stage < 5:
        dbg = nc.dram_tensor("dbg", [R, D_OUT], F32, kind="ExternalOutput").ap()

    with tile.TileContext(nc) as tc, ExitStack() as ctx:
        # assemble full Wh/Wl on every core via AllGather over NeuronLink
        nc.sync.dma_start(wstg[:], whl[:])
        nc.gpsimd.collective_compute(
            "AllGather", AluOpType.bypass,
            replica_groups=[[i for i in range(N_CORES)]],
            ins=[wstg[:]], outs=[wgth[:]],
        )

        wp = ctx.enter_context(tc.tile_pool(name="w", bufs=1))
        xp = ctx.enter_context(tc.tile_pool(name="x", bufs=2))
        hp = ctx.enter_context(tc.tile_pool(name="h", bufs=2))
        kp = ctx.enter_context(tc.tile_pool(name="k", bufs=2))
        cp = ctx.enter_context(tc.tile_pool(name="c", bufs=2))
        sp = ctx.enter_context(tc.tile_pool(name="s", bufs=2))
        pp = ctx.enter_context(tc.tile_pool(name="ps", bufs=6, space="PSUM"))
        tp = ctx.enter_context(tc.tile_pool(name="tp", bufs=2, space="PSUM"))

        # resident Wh/Wl (fp16), k-chunk-major layout:
        #   w?[:, k*D_OUT + c*512 : ...] = W?[k*128:(k+1)*128, c*512:(c+1)*512]
        wh = wp.tile([P, KC * D_OUT], F16, tag="wh")
        wl = wp.tile([P, KC * D_OUT], F16, tag="wl")
        for k in range(KC):
            for c in range(N_CORES):
                base = c * 2 * D_IN
                nc.sync.dma_start(
                    wh[:, k * D_OUT + c * WSL: k * D_OUT + (c + 1) * WSL],
                    wgth[base + k * P: base + (k + 1) * P, :])
                nc.sync.dma_start(
                    wl[:, k * D_OUT + c * WSL: k * D_OUT + (c + 1) * WSL],
                    wgth[base + D_IN + k * P: base + D_IN + (k + 1) * P, :])

        ident = wp.tile([P, P], F16, tag="ident")
        masks.make_identity(nc, ident[:])
        c31 = wp.tile([P, 1], I32, tag="c31")
        nc.vector.memset(c31[:], 31)
        cmask = wp.tile([P, 1], I32, tag="cmask")
        nc.vector.memset(cmask[:], 0x7FFFFFFE)

        for it in range(n_tiles):
            xa = xp.tile([P, D_IN], F16, tag="xa")
            nc.sync.dma_start(xa[:], x16[it * P:(it + 1) * P, :])
            x8 = xp.tile([P, D_IN], F8, tag="x8")
            nc.sync.dma_start(x8[:], xl8[it * P:(it + 1) * P, :])
            xl = xp.tile([P, D_IN], F16, tag="xl")
            nc.scalar.activation(xl[:], x8[:], AF.Copy, scale=1.0 / XL_SCALE)

            # xT chunks [128 k-part, 128 rows] via PE transpose (hi then lo)
            xT = xp.tile([P, 2 * KC * P], F16, tag="xT")
            for j, src in ((0, xa), (1, xl)):
                for k in range(KC):
                    tps = tp.tile([P, P], F16, tag="tps")
                    nc.tensor.transpose(tps[:], src[:, k * P:(k + 1) * P],
                                        ident[:])
                    nc.scalar.copy(
                        xT[:, (j * KC + k) * P:(j * KC + k + 1) * P], tps[:])

            hs = hp.tile([P, D_OUT], F32, tag="hs")
            sparts = sp.tile([P, NBANK], F32, tag="sparts")
            n_mm = 3 * KC
            for b in range(NBANK):
                ps = pp.tile([P, 512], F32, tag="ps")
                i = 0
                for k in range(KC):
                    xh_k = xT[:, k * P:(k + 1) * P]
                    xl_k = xT[:, (KC + k) * P:(KC + k + 1) * P]
                    ws = slice(k * D_OUT + b * 512, k * D_OUT + (b + 1) * 512)
                    for lhs, rhs in ((xh_k, wh), (xh_k, wl), (xl_k, wh)):
                        nc.tensor.matmul(ps[:], lhs, rhs[:, ws],
                                         start=(i == 0), stop=(i == n_mm - 1))
                        i += 1
                nc.scalar.activation(hs[:, b * 512:(b + 1) * 512], ps[:],
                                     AF.Copy, accum_out=sparts[:, b:b + 1])

            if stage <= 1:
                nc.sync.dma_start(dbg[it * P:(it + 1) * P, :], hs[:])
                continue

            ssum = sp.tile([P, 1], F32, tag="ssum")
            nc.vector.reduce_sum(ssum[:], sparts[:], axis=AX.X)
            negmu = sp.tile([P, 1], F32, tag="negmu")
            nc.vector.tensor_scalar(out=negmu[:], in0=ssum[:],
                                    scalar1=-1.0 / D_OUT, scalar2=None,
                                    op0=AluOpType.mult)
            # hm = h - mu  (in place)
            nc.scalar.activation(hs[:], hs[:], AF.Identity, bias=negmu[:],
                                 scale=1.0)

            # pack: (bits(hm) & 0x7FFFFFFE) | sign  -> |hm| with sign in bit 0
            pk = kp.tile([P, D_OUT], I32, tag="pk")
            u = hs[:].bitcast(I32)
            nc.vector.tensor_scalar(out=pk[:], in0=u, scalar1=c31[:],
                                    scalar2=None,
                                    op0=AluOpType.logical_shift_right)
            nc.vector.scalar_tensor_tensor(out=pk[:], in0=u, scalar=cmask[:],
                                           in1=pk[:],
                                           op0=AluOpType.bitwise_and,
                                           op1=AluOpType.bitwise_or)
            pf = pk[:].bitcast(F32)

            if stage <= 2:
                nc.sync.dma_start(dbg[it * P:(it + 1) * P, :], pf)
                continue

            # L1: per-segment top-8 candidates (packed words)
            cand = cp.tile([P, NSEG * 8], F32, tag="cand")
            for s in range(NSEG):
                nc.vector.max(cand[:, s * 8:(s + 1) * 8],
                              pf[:, s * SEG:(s + 1) * SEG])

            # L2: 8 rounds of max8 + match_replace -> top-64 packed words
            vals = cp.tile([P, K], F32, tag="vals")
            mr = cp.tile([P, NSEG * 8], F32, tag="mr")
            cur = cand
            for r in range(K // 8):
                nc.vector.max(vals[:, r * 8:(r + 1) * 8], cur[:])
                if r < K // 8 - 1:
                    nxt = mr if cur is cand else cand
                    nc.vector.match_replace(nxt[:], vals[:, r * 8:(r + 1) * 8],
                                            cur[:], NEG)
                    cur = nxt

            # recover global column indices of the selected packed words
            vidx = cp.tile([P, K], U16, tag="vidx")
            for r in range(K // 8):
                nc.vector.max_index(vidx[:, r * 8:(r + 1) * 8],
                                    vals[:, r * 8:(r + 1) * 8], pf)

            nc.sync.dma_start(vout[it * P:(it + 1) * P, :], vals[:])
            nc.sync.dma_start(iout[it * P:(it + 1) * P, :], vidx[:])

    nc.compile()
    return nc


def _get_nc():
    if "nc" not in _CACHE:
        _CACHE["nc"] = _build()
    return _CACHE["nc"]


def _get_runner(nc):
    """Cached jit of the SPMD execute (mirrors bass2jax.run_bass_via_pjrt,
    but reused across calls so repeat calls skip retrace/recompile)."""
    if "runner" in _CACHE:
        return _CACHE["runner"]
    import jax
    from jax.experimental.shard_map import shard_map
    from jax.sharding import Mesh, PartitionSpec
    from concourse import bass2jax as b2j

    b2j.install_neuronx_cc_hook()

    partition_name = (nc.partition_id_tensor.name
                      if nc.partition_id_tensor else None)
    in_names, out_names, out_avals, zero_outs = [], [], [], []
    for alloc in nc.m.functions[0].allocations:
        if not isinstance(alloc, mybir.MemoryLocationSet):
            continue
        name = alloc.memorylocations[0].name
        if alloc.kind == "ExternalInput":
            if name != partition_name:
                in_names.append(name)
        elif alloc.kind == "ExternalOutput":
            shape = tuple(alloc.tensor_shape)
            dtype = mybir.dt.np(alloc.dtype)
            out_names.append(name)
            out_avals.append(jax.core.ShapedArray(shape, dtype))
            zero_outs.append(np.zeros((N_CORES * shape[0], *shape[1:]), dtype))
    n_params = len(in_names)
    n_outs = len(out_avals)
    all_names = list(in_names) + list(out_names)
    if partition_name is not None:
        all_names.append(partition_name)
    donate = tuple(range(n_params, n_params + n_outs))

    def _body(*args):
        operands = list(args)
        if partition_name is not None:
            operands.append(b2j.partition_id_tensor())
        outs = b2j._bass_exec_p.bind(
            *operands,
            out_avals=tuple(out_avals),
            in_names=tuple(all_names),
            out_names=tuple(out_names),
            lowering_input_output_aliases=(),
            sim_require_finite=True,
            sim_require_nnan=True,
            nc=nc,
        )
        return tuple(outs)

    mesh = Mesh(np.asarray(jax.devices()[:N_CORES]), ("core",))
    specs = (PartitionSpec("core"),)
    sharded = jax.jit(
        shard_map(_body, mesh=mesh,
                  in_specs=specs * (n_params + n_outs),
                  out_specs=specs * n_outs,
                  check_rep=False),
        donate_argnums=donate, keep_unused=True)
    runner = (sharded, in_names, out_names, out_avals, zero_outs)
    _CACHE["runner"] = runner
    return runner


def _run_spmd(nc, in_maps):
    """Returns list (per core) of {out_name: np.ndarray}."""
    try:
        sharded, in_names, out_names, out_avals, zero_outs = _get_runner(nc)
        concat_in = [
            np.concatenate([np.asarray(in_maps[c][name])
                            for c in range(N_CORES)], axis=0)
            for name in in_names
        ]
        out_arrs = sharded(*concat_in, *zero_outs)
        # donated zeros were consumed; rebuild for the next call
        _CACHE["runner"] = (sharded, in_names, out_names, out_avals, [
            np.zeros((N_CORES * a.shape[0], *a.shape[1:]), a.dtype)
            for a in out_avals
        ])
        return [
            {name: np.asarray(out_arrs[i]).reshape(
                N_CORES, *out_avals[i].shape)[c]
             for i, name in enumerate(out_names)}
            for c in range(N_CORES)
        ]
    except Exception:
        _CACHE.pop("runner", None)
        res = bass_utils.run_bass_kernel_spmd(
            nc, in_maps, core_ids=list(range(N_CORES)))
        return [res.results[c] for c in range(N_CORES)]


def _numpy_fallback(x, W, b, gamma, beta):
    h = x.astype(np.float32) @ W.astype(np.float32) + b
    mu = h.mean(-1, keepdims=True)
    var = np.square(h - mu).mean(-1, keepdims=True)
    p = (h - mu) / np.sqrt(var + 1e-5) * gamma + beta
    idx = np.argsort(-np.abs(p), axis=-1, kind="stable")[:, :K]
    sparse = np.zeros_like(p)
    np.put_along_axis(sparse, idx, np.take_along_axis(p, idx, -1), -1)
    nrm = np.linalg.norm(sparse, axis=-1, keepdims=True)
    return sparse / np.maximum(nrm, 1e-12)


def _repair_rows(dense, rows, x, W):
    """Exact fp32 recompute of the given rows (b=0/beta=0/gamma=1 math)."""
    if len(rows) == 0:
        return
    h = x[rows].astype(np.float32) @ W.astype(np.float32)
    hm = h - h.mean(-1, keepdims=True, dtype=np.float32)
    idx = np.argsort(-np.abs(hm), axis=-1, kind="stable")[:, :K]
    vals = np.take_along_axis(hm, idx, -1)
    nrm = np.maximum(np.linalg.norm(vals, axis=-1, keepdims=True), 1e-12)
    block = np.zeros((len(rows), D_OUT), np.float32)
    np.put_along_axis(block, idx, vals / nrm, -1)
    dense[rows] = block


def kernel(**inputs):
    x = np.asarray(inputs["x"], dtype=np.float32)
    W = np.asarray(inputs["W"], dtype=np.float32)
    b = np.asarray(inputs["b"], dtype=np.float32)
    gamma = np.asarray(inputs["gamma"], dtype=np.float32)
    beta = np.asarray(inputs["beta"], dtype=np.float32)

    # kernel math relies on b == 0, beta == 0, gamma == const > 0 (per spec)
    if (np.any(b != 0) or np.any(beta != 0)
            or np.any(gamma != gamma[0]) or gamma[0] <= 0):
        return _numpy_fallback(x, W, b, gamma, beta)

    x16 = x.astype(np.float16)
    xl8 = ((x - x16) * XL_SCALE).astype(ml_dtypes.float8_e4m3)
    Wh = W.astype(np.float16)
    Wl = (W - Wh).astype(np.float16)
    in_maps = [
        {"x16": x16[c * R:(c + 1) * R],
         "xl8": xl8[c * R:(c + 1) * R],
         "whl": np.concatenate([Wh[:, c * WSL:(c + 1) * WSL],
                                Wl[:, c * WSL:(c + 1) * WSL]], axis=0)}
        for c in range(N_CORES)
    ]
    nc = _get_nc()
    res = _run_spmd(nc, in_maps)
    vals = np.concatenate([res[c]["vout"] for c in range(N_CORES)], axis=0)
    vidx = np.concatenate([res[c]["iout"] for c in range(N_CORES)], axis=0)

    # decode: sign in bit 0, magnitude in the remaining bits
    bits = np.ascontiguousarray(vals).view(np.int32)        # [B, K]
    sgn = (bits & 1).astype(np.float32)
    mag = (bits & np.int32(-2)).view(np.float32)
    val = mag * (1.0 - 2.0 * sgn)
    idx = vidx.astype(np.intp)
    nrm = np.sqrt(np.sum(np.square(mag, dtype=np.float64), axis=1,
                         keepdims=True))
    nrm = np.maximum(nrm, 1e-12)
    dense = np.zeros((B, D_OUT), np.float32)
    np.put_along_axis(dense, idx, (val / nrm).astype(np.float32), axis=-1)

    # rows where a duplicated |h-mu| made max_index return the same column
    # twice (or an index escaped [0, D_OUT)): recompute exactly on host
    srt = np.sort(idx, axis=1)
    bad = (srt[:, 1:] == srt[:, :-1]).any(axis=1) | (idx >= D_OUT).any(axis=1)
    _repair_rows(dense, np.flatnonzero(bad), x, W)
    return dense


# revision 6
# speedup vs baseline: 5.5329x; 1.1400x over previous
"""ContrastiveSparseRepresentation TRN2 kernel.

out = normalize(topk_mask(layernorm(x @ W + b) * gamma + beta, k=64))

Math used (valid for b=0, beta=0, gamma=const>0, per the problem spec):
  p = (h - mu) * rsqrt(var + eps) * g;  topk by |p| == topk by |h - mu|;
  normalize(mask * p) == mask * (h - mu) / ||mask * (h - mu)||  (g, rsqrt cancel)

Wall-clock here is dominated by host<->device transfer over the axon tunnel
(~100 MB/s), so the kernel minimizes bytes moved:
  - x is uploaded as fp16 (50 MB) plus an fp8-e4m3 low-order correction
    (x - fp16(x)) * 2048 (25 MB).  On device the correction is scaled back
    and a 3-term fp16 matmul (xh*Wh + xh*Wl + xl*Wh) reproduces fp32-level
    accuracy (h err ~1e-5), which keeps top-64 support flips to ~25 rows.
  - W is uploaded pre-split into fp16 hi/lo halves, sharded by column
    across the 8 cores (1.6 MB/core), and assembled on-device with an
    HBM-to-HBM AllGather: 12.6 MB total instead of 100 MB replicated.
  - The device returns only the top-64 per row: packed values [R,64] f32
    (|h-mu| with sign stuffed into mantissa bit 0) + column indices [R,64]
    u16 recovered with DVE max_index.  Download is 12 MB instead of 512 MB.
    The host decodes, normalizes, scatters to dense, and exactly recomputes
    the rare rows where a duplicated |h-mu| value made max_index return the
    same column twice.

Sharding: data-parallel over the 32768-row batch across 8 NeuronCores.
Per core: 4096 rows = 32 tiles of 128 rows (partition dim).
"""

import numpy as np
from contextlib import ExitStack

import ml_dtypes
import concourse.bass as bass
import concourse.tile as tile
from concourse import bacc, masks, mybir
from concourse import bass_utils
from concourse.alu_op_type import AluOpType

F32 = mybir.dt.float32
F16 = mybir.dt.float16
F8 = mybir.dt.float8e4
I32 = mybir.dt.int32
U16 = mybir.dt.uint16
AF = mybir.ActivationFunctionType
AX = mybir.AxisListType

B, D_IN, D_OUT = 32768, 768, 4096
N_CORES = 8
R = B // N_CORES            # rows per core
P = 128                     # rows per tile (partition dim)
N_TILES = R // P            # 32
KC = D_IN // P              # 6 contraction chunks
NBANK = D_OUT // 512        # 8 psum banks
WSL = D_OUT // N_CORES      # 512: W columns owned per core
SEG = 64
NSEG = D_OUT // SEG         # 64 segments
K = 64                      # top-k
NEG = -1e30
XL_SCALE = 2048.0           # 2**11: fp8 correction prescale

_CACHE = {}


def _build(n_tiles=N_TILES, stage=5):
    nc = bacc.Bacc("TRN2", target_bir_lowering=False, debug=False,
                   num_devices=N_CORES, enable_asserts=False)
    x16 = nc.dram_tensor("x16", [R, D_IN], F16, kind="ExternalInput").ap()
    xl8 = nc.dram_tensor("xl8", [R, D_IN], F8, kind="ExternalInput").ap()
    # [Wh ; Wl] fp16, this core's 512-column slice
    whl = nc.dram_tensor("whl", [2 * D_IN, WSL], F16,
                         kind="ExternalInput").ap()
    wstg = nc.dram_tensor("wstg", [2 * D_IN, WSL], F16, kind="Internal").ap()
    wgth = nc.dram_tensor("wgth", [N_CORES * 2 * D_IN, WSL], F16,
                          kind="Internal").ap()
    vout = nc.dram_tensor("vout", [R, K], F32, kind="ExternalOutput").ap()
    iout = nc.dram_tensor("iout", [R, K], U16, kind="ExternalOutput").ap()
    dbg = None
    if stage < 5:
        dbg = nc.dram_tensor("dbg", [R, D_OUT], F32, kind="ExternalOutput").ap()

    with tile.TileContext(nc) as tc, ExitStack() as ctx:
        # assemble full Wh/Wl on every core via AllGather over NeuronLink
        nc.sync.dma_start(wstg[:], whl[:])
        nc.gpsimd.collective_compute(
            "AllGather", AluOpType.bypass,
            replica_groups=[[i for i in range(N_CORES)]],
            ins=[wstg[:]], outs=[wgth[:]],
        )

        wp = ctx.enter_context(tc.tile_pool(name="w", bufs=1))
        xp = ctx.enter_context(tc.tile_pool(name="x", bufs=2))
        hp = ctx.enter_context(tc.tile_pool(name="h", bufs=2))
        kp = ctx.enter_context(tc.tile_pool(name="k", bufs=2))
        cp = ctx.enter_context(tc.tile_pool(name="c", bufs=2))
        sp = ctx.enter_context(tc.tile_pool(name="s", bufs=2))
        pp = ctx.enter_context(tc.tile_pool(name="ps", bufs=6, space="PSUM"))
        tp = ctx.enter_context(tc.tile_pool(name="tp", bufs=2, space="PSUM"))

        # resident Wh/Wl (fp16), k-chunk-major layout:
        #   w?[:, k*D_OUT + c*512 : ...] = W?[k*128:(k+1)*128, c*512:(c+1)*512]
        wh = wp.tile([P, KC * D_OUT], F16, tag="wh")
        wl = wp.tile([P, KC * D_OUT], F16, tag="wl")
        for k in range(KC):
            for c in range(N_CORES):
                base = c * 2 * D_IN
                nc.sync.dma_start(
                    wh[:, k * D_OUT + c * WSL: k * D_OUT + (c + 1) * WSL],
                    wgth[base + k * P: base + (k + 1) * P, :])
                nc.sync.dma_start(
                    wl[:, k * D_OUT + c * WSL: k * D_OUT + (c + 1) * WSL],
                    wgth[base + D_IN + k * P: base + D_IN + (k + 1) * P, :])

        ident = wp.tile([P, P], F16, tag="ident")
        masks.make_identity(nc, ident[:])
        c31 = wp.tile([P, 1], I32, tag="c31")
        nc.vector.memset(c31[:], 31)
        cmask = wp.tile([P, 1], I32, tag="cmask")
        nc.vector.memset(cmask[:], 0x7FFFFFFE)

        for it in range(n_tiles):
            xa = xp.tile([P, D_IN], F16, tag="xa")
            nc.sync.dma_start(xa[:], x16[it * P:(it + 1) * P, :])
            x8 = xp.tile([P, D_IN], F8, tag="x8")
            nc.sync.dma_start(x8[:], xl8[it * P:(it + 1) * P, :])
            xl = xp.tile([P, D_IN], F16, tag="xl")
            nc.scalar.activation(xl[:], x8[:], AF.Copy, scale=1.0 / XL_SCALE)

            # xT chunks [128 k-part, 128 rows] via PE transpose (hi then lo)
            xT = xp.tile([P, 2 * KC * P], F16, tag="xT")
            for j, src in ((0, xa), (1, xl)):
                for k in range(KC):
                    tps = tp.tile([P, P], F16, tag="tps")
                    nc.tensor.transpose(tps[:], src[:, k * P:(k + 1) * P],
                                        ident[:])
                    nc.scalar.copy(
                        xT[:, (j * KC + k) * P:(j * KC + k + 1) * P], tps[:])

            hs = hp.tile([P, D_OUT], F32, tag="hs")
            sparts = sp.tile([P, NBANK], F32, tag="sparts")
            n_mm = 3 * KC
            for b in range(NBANK):
                ps = pp.tile([P, 512], F32, tag="ps")
                i = 0
                for k in range(KC):
                    xh_k = xT[:, k * P:(k + 1) * P]
                    xl_k = xT[:, (KC + k) * P:(KC + k + 1) * P]
                    ws = slice(k * D_OUT + b * 512, k * D_OUT + (b + 1) * 512)
                    for lhs, rhs in ((xh_k, wh), (xh_k, wl), (xl_k, wh)):
                        nc.tensor.matmul(ps[:], lhs, rhs[:, ws],
                                         start=(i == 0), stop=(i == n_mm - 1))
                        i += 1
                nc.scalar.activation(hs[:, b * 512:(b + 1) * 512], ps[:],
                                     AF.Copy, accum_out=sparts[:, b:b + 1])

            if stage <= 1:
                nc.sync.dma_start(dbg[it * P:(it + 1) * P, :], hs[:])
                continue

            ssum = sp.tile([P, 1], F32, tag="ssum")
            nc.vector.reduce_sum(ssum[:], sparts[:], axis=AX.X)
            negmu = sp.tile([P, 1], F32, tag="negmu")
            nc.vector.tensor_scalar(out=negmu[:], in0=ssum[:],
                                    scalar1=-1.0 / D_OUT, scalar2=None,
                                    op0=AluOpType.mult)
            # hm = h - mu  (in place)
            nc.scalar.activation(hs[:], hs[:], AF.Identity, bias=negmu[:],
                                 scale=1.0)

            # pack: (bits(hm) & 0x7FFFFFFE) | sign  -> |hm| with sign in bit 0
            pk = kp.tile([P, D_OUT], I32, tag="pk")
            u = hs[:].bitcast(I32)
            nc.vector.tensor_scalar(out=pk[:], in0=u, scalar1=c31[:],
                                    scalar2=None,
                                    op0=AluOpType.logical_shift_right)
            nc.vector.scalar_tensor_tensor(out=pk[:], in0=u, scalar=cmask[:],
                                           in1=pk[:],
                                           op0=AluOpType.bitwise_and,
                                           op1=AluOpType.bitwise_or)
            pf = pk[:].bitcast(F32)

            if stage <= 2:
                nc.sync.dma_start(dbg[it * P:(it + 1) * P, :], pf)
                continue

            # L1: per-segment top-8 candidates (packed words)
            cand = cp.tile([P, NSEG * 8], F32, tag="cand")
            for s in range(NSEG):
                nc.vector.max(cand[:, s * 8:(s + 1) * 8],
                              pf[:, s * SEG:(s + 1) * SEG])

            # L2: 8 rounds of max8 + match_replace -> top-64 packed words
            vals = cp.tile([P, K], F32, tag="vals")
            mr = cp.tile([P, NSEG * 8], F32, tag="mr")
            cur = cand
            for r in range(K // 8):
                nc.vector.max(vals[:, r * 8:(r + 1) * 8], cur[:])
                if r < K // 8 - 1:
                    nxt = mr if cur is cand else cand
                    nc.vector.match_replace(nxt[:], vals[:, r * 8:(r + 1) * 8],
                                            cur[:], NEG)
                    cur = nxt

            # recover global column indices of the selected packed words
            vidx = cp.tile([P, K], U16, tag="vidx")
            for r in range(K // 8):
                nc.vector.max_index(vidx[:, r * 8:(r + 1) * 8],
                                    vals[:, r * 8:(r + 1) * 8], pf)

            nc.sync.dma_start(vout[it * P:(it + 1) * P, :], vals[:])
            nc.sync.dma_start(iout[it * P:(it + 1) * P, :], vidx[:])

    nc.compile()
    return nc


def _get_nc():
    if "nc" not in _CACHE:
        _CACHE["nc"] = _build()
    return _CACHE["nc"]


def _get_runner(nc):
    """Cached jit of the SPMD execute (mirrors bass2jax.run_bass_via_pjrt,
    but reused across calls so repeat calls skip retrace/recompile)."""
    if "runner" in _CACHE:
        return _CACHE["runner"]
    import jax
    from jax.experimental.shard_map import shard_map
    from jax.sharding import Mesh, PartitionSpec
    from concourse import bass2jax as b2j

    b2j.install_neuronx_cc_hook()

    partition_name = (nc.partition_id_tensor.name
                      if nc.partition_id_tensor else None)
    in_names, out_names, out_avals, zero_outs = [], [], [], []
    for alloc in nc.m.functions[0].allocations:
        if not isinstance(alloc, mybir.MemoryLocationSet):
            continue
        name = alloc.memorylocations[0].name
        if alloc.kind == "ExternalInput":
            if name != partition_name:
                in_names.append(name)
        elif alloc.kind == "ExternalOutput":
            shape = tuple(alloc.tensor_shape)
            dtype = mybir.dt.np(alloc.dtype)
            out_names.append(name)
            out_avals.append(jax.core.ShapedArray(shape, dtype))
            zero_outs.append(np.zeros((N_CORES * shape[0], *shape[1:]), dtype))
    n_params = len(in_names)
    n_outs = len(out_avals)
    all_names = list(in_names) + list(out_names)
    if partition_name is not None:
        all_names.append(partition_name)
    donate = tuple(range(n_params, n_params + n_outs))

    def _body(*args):
        operands = list(args)
        if partition_name is not None:
            operands.append(b2j.partition_id_tensor())
        outs = b2j._bass_exec_p.bind(
            *operands,
            out_avals=tuple(out_avals),
            in_names=tuple(all_names),
            out_names=tuple(out_names),
            lowering_input_output_aliases=(),
            sim_require_finite=True,
            sim_require_nnan=True,
            nc=nc,
        )
        return tuple(outs)

    mesh = Mesh(np.asarray(jax.devices()[:N_CORES]), ("core",))
    specs = (PartitionSpec("core"),)
    sharded = jax.jit(
        shard_map(_body, mesh=mesh,
                  in_specs=specs * (n_params + n_outs),
                  out_specs=specs * n_outs,
                  check_rep=False),
        donate_argnums=donate, keep_unused=True)
    runner = (sharded, in_names, out_names, out_avals, zero_outs)
    _CACHE["runner"] = runner
    return runner


def _run_spmd(nc, in_maps):
    """Returns list (per core) of {out_name: np.ndarray}."""
    try:
        sharded, in_names, out_names, out_avals, zero_outs = _get_runner(nc)
        concat_in = [
            np.concatenate([np.asarray(in_maps[c][name])
                            for c in range(N_CORES)], axis=0)
            for name in in_names
        ]
        out_arrs = sharded(*concat_in, *zero_outs)
        # donated zeros were consumed; rebuild for the next call
        _CACHE["runner"] = (sharded, in_names, out_names, out_avals, [
            np.zeros((N_CORES * a.shape[0], *a.shape[1:]), a.dtype)
            for a in out_avals
        ])
        return [
            {name: np.asarray(out_arrs[i]).reshape(
                N_CORES, *out_avals[i].shape)[c]
             for i, name in enumerate(out_names)}
            for c in range(N_CORES)
        ]
    except Exception:
        _CACHE.pop("runner", None)
        res = bass_utils.run_bass_kernel_spmd(
            nc, in_maps, core_ids=list(range(N_CORES)))
        return [res.results[c] for c in range(N_CORES)]


def _numpy_fallback(x, W, b, gamma, beta):
    h = x.astype(np.float32) @ W.astype(np.float32) + b
    mu = h.mean(-1, keepdims=True)
    var = np.square(h - mu).mean(-1, keepdims=True)
    p = (h - mu) / np.sqrt(var + 1e-5) * gamma + beta
    idx = np.argsort(-np.abs(p), axis=-1, kind="stable")[:, :K]
    sparse = np.zeros_like(p)
    np.put_along_axis(sparse, idx, np.take_along_axis(p, idx, -1), -1)
    nrm = np.linalg.norm(sparse, axis=-1, keepdims=True)
    return sparse / np.maximum(nrm, 1e-12)


def _repair_rows(dense, rows, x, W):
    """Exact fp32 recompute of the given rows (b=0/beta=0/gamma=1 math)."""
    if len(rows) == 0:
        return
    h = x[rows].astype(np.float32) @ W.astype(np.float32)
    hm = h - h.mean(-1, keepdims=True, dtype=np.float32)
    idx = np.argsort(-np.abs(hm), axis=-1, kind="stable")[:, :K]
    vals = np.take_along_axis(hm, idx, -1)
    nrm = np.maximum(np.linalg.norm(vals, axis=-1, keepdims=True), 1e-12)
    block = np.zeros((len(rows), D_OUT), np.float32)
    np.put_along_axis(block, idx, vals / nrm, -1)
    dense[rows] = block


def kernel(**inputs):
    import os, time
    prof = os.environ.get("KERNEL_PROF") == "1"
    tt = time.time
    t0 = tt()
    x = np.asarray(inputs["x"], dtype=np.float32)
    W = np.asarray(inputs["W"], dtype=np.float32)
    b = np.asarray(inputs["b"], dtype=np.float32)
    gamma = np.asarray(inputs["gamma"], dtype=np.float32)
    beta = np.asarray(inputs["beta"], dtype=np.float32)

    # kernel math relies on b == 0, beta == 0, gamma == const > 0 (per spec)
    if (np.any(b != 0) or np.any(beta != 0)
            or np.any(gamma != gamma[0]) or gamma[0] <= 0):
        return _numpy_fallback(x, W, b, gamma, beta)

    x16 = x.astype(np.float16)
    xl8 = ((x - x16) * XL_SCALE).astype(ml_dtypes.float8_e4m3)
    Wh = W.astype(np.float16)
    Wl = (W - Wh).astype(np.float16)
    in_maps = [
        {"x16": x16[c * R:(c + 1) * R],
         "xl8": xl8[c * R:(c + 1) * R],
         "whl": np.concatenate([Wh[:, c * WSL:(c + 1) * WSL],
                                Wl[:, c * WSL:(c + 1) * WSL]], axis=0)}
        for c in range(N_CORES)
    ]
    t1 = tt()
    nc = _get_nc()
    t2 = tt()
    res = _run_spmd(nc, in_maps)
    t3 = tt()
    vals = np.concatenate([res[c]["vout"] for c in range(N_CORES)], axis=0)
    vidx = np.concatenate([res[c]["iout"] for c in range(N_CORES)], axis=0)

    # decode: sign in bit 0, magnitude in the remaining bits
    bits = np.ascontiguousarray(vals).view(np.int32)        # [B, K]
    sgn = (bits & 1).astype(np.float32)
    mag = (bits & np.int32(-2)).view(np.float32)
    val = mag * (1.0 - 2.0 * sgn)
    idx = vidx.astype(np.intp)
    nrm = np.sqrt(np.sum(np.square(mag, dtype=np.float64), axis=1,
                         keepdims=True))
    nrm = np.maximum(nrm, 1e-12)
    dense = np.zeros((B, D_OUT), np.float32)
    np.put_along_axis(dense, idx, (val / nrm).astype(np.float32), axis=-1)

    # rows where a duplicated |h-mu| made max_index return the same column
    # twice (or an index escaped [0, D_OUT)): recompute exactly on host
    srt = np.sort(idx, axis=1)
    bad = (srt[:, 1:] == srt[:, :-1]).any(axis=1) | (idx >= D_OUT).any(axis=1)
    _repair_rows(dense, np.flatnonzero(bad), x, W)
    if prof:
        t4 = tt()
        print(f"[prof] prep:{t1 - t0:.2f} nc:{t2 - t1:.2f} "
              f"run:{t3 - t2:.2f} decode:{t4 - t3:.2f} "
              f"bad_rows:{int(bad.sum())}")
    return dense


# revision 8
# speedup vs baseline: 12.1741x; 2.2003x over previous
"""ContrastiveSparseRepresentation TRN2 kernel.

out = normalize(topk_mask(layernorm(x @ W + b) * gamma + beta, k=64))

Math used (valid for b=0, beta=0, gamma=const>0, per the problem spec):
  p = (h - mu) * rsqrt(var + eps) * g;  topk by |p| == topk by |h - mu|;
  normalize(mask * p) == mask * (h - mu) / ||mask * (h - mu)||  (g, rsqrt cancel)

Wall-clock here is dominated by host<->device transfer over the axon tunnel
(~100 MB/s), so the kernel minimizes bytes moved:
  - x is uploaded as fp16 (50 MB) plus an fp8-e4m3 low-order correction
    (x - fp16(x)) * 2048 (25 MB).  On device the correction is scaled back
    and a 3-term fp16 matmul (xh*Wh + xh*Wl + xl*Wh) reproduces fp32-level
    accuracy (h err ~1e-5), which keeps top-64 support flips to ~25 rows.
  - W is uploaded pre-split into fp16 hi/lo halves, sharded by column
    across the 8 cores (1.6 MB/core), and assembled on-device with an
    HBM-to-HBM AllGather: 12.6 MB total instead of 100 MB replicated.
  - The device returns only the top-64 per row: packed values [R,64] f32
    (|h-mu| with sign stuffed into mantissa bit 0) + column indices [R,64]
    u16 recovered with DVE max_index.  Download is 12 MB instead of 512 MB.
    The host decodes, normalizes, scatters to dense, and exactly recomputes
    the rare rows where a duplicated |h-mu| value made max_index return the
    same column twice.

Sharding: data-parallel over the 32768-row batch across 8 NeuronCores.
Per core: 4096 rows = 32 tiles of 128 rows (partition dim).
"""

import numpy as np
from contextlib import ExitStack

import ml_dtypes
import concourse.bass as bass
import concourse.tile as tile
from concourse import bacc, masks, mybir
from concourse import bass_utils
from concourse.alu_op_type import AluOpType

F32 = mybir.dt.float32
F16 = mybir.dt.float16
F8 = mybir.dt.float8e4
I32 = mybir.dt.int32
U16 = mybir.dt.uint16
AF = mybir.ActivationFunctionType
AX = mybir.AxisListType

B, D_IN, D_OUT = 32768, 768, 4096
N_CORES = 8
R = B // N_CORES            # rows per core
P = 128                     # rows per tile (partition dim)
N_TILES = R // P            # 32
KC = D_IN // P              # 6 contraction chunks
NBANK = D_OUT // 512        # 8 psum banks
WSL = D_OUT // N_CORES      # 512: W columns owned per core
SEG = 64
NSEG = D_OUT // SEG         # 64 segments
K = 64                      # top-k
NEG = -1e30
XL_SCALE = 2048.0           # 2**11: fp8 correction prescale

_CACHE = {}


def _build(n_tiles=N_TILES, stage=5):
    nc = bacc.Bacc("TRN2", target_bir_lowering=False, debug=False,
                   num_devices=N_CORES, enable_asserts=False)
    x16 = nc.dram_tensor("x16", [R, D_IN], F16, kind="ExternalInput").ap()
    xl8 = nc.dram_tensor("xl8", [R, D_IN], F8, kind="ExternalInput").ap()
    # [Wh ; Wl] fp16, this core's 512-column slice
    whl = nc.dram_tensor("whl", [2 * D_IN, WSL], F16,
                         kind="ExternalInput").ap()
    wstg = nc.dram_tensor("wstg", [2 * D_IN, WSL], F16, kind="Internal").ap()
    wgth = nc.dram_tensor("wgth", [N_CORES * 2 * D_IN, WSL], F16,
                          kind="Internal").ap()
    vout = nc.dram_tensor("vout", [R, K], F32, kind="ExternalOutput").ap()
    iout = nc.dram_tensor("iout", [R, K], U16, kind="ExternalOutput").ap()
    dbg = None
    if stage < 5:
        dbg = nc.dram_tensor("dbg", [R, D_OUT], F32, kind="ExternalOutput").ap()

    with tile.TileContext(nc) as tc, ExitStack() as ctx:
        # assemble full Wh/Wl on every core via AllGather over NeuronLink
        nc.sync.dma_start(wstg[:], whl[:])
        nc.gpsimd.collective_compute(
            "AllGather", AluOpType.bypass,
            replica_groups=[[i for i in range(N_CORES)]],
            ins=[wstg[:]], outs=[wgth[:]],
        )

        wp = ctx.enter_context(tc.tile_pool(name="w", bufs=1))
        xp = ctx.enter_context(tc.tile_pool(name="x", bufs=2))
        hp = ctx.enter_context(tc.tile_pool(name="h", bufs=2))
        kp = ctx.enter_context(tc.tile_pool(name="k", bufs=2))
        cp = ctx.enter_context(tc.tile_pool(name="c", bufs=2))
        sp = ctx.enter_context(tc.tile_pool(name="s", bufs=2))
        pp = ctx.enter_context(tc.tile_pool(name="ps", bufs=6, space="PSUM"))
        tp = ctx.enter_context(tc.tile_pool(name="tp", bufs=2, space="PSUM"))

        # resident Wh/Wl (fp16), k-chunk-major layout:
        #   w?[:, k*D_OUT + c*512 : ...] = W?[k*128:(k+1)*128, c*512:(c+1)*512]
        wh = wp.tile([P, KC * D_OUT], F16, tag="wh")
        wl = wp.tile([P, KC * D_OUT], F16, tag="wl")
        for k in range(KC):
            for c in range(N_CORES):
                base = c * 2 * D_IN
                nc.sync.dma_start(
                    wh[:, k * D_OUT + c * WSL: k * D_OUT + (c + 1) * WSL],
                    wgth[base + k * P: base + (k + 1) * P, :])
                nc.sync.dma_start(
                    wl[:, k * D_OUT + c * WSL: k * D_OUT + (c + 1) * WSL],
                    wgth[base + D_IN + k * P: base + D_IN + (k + 1) * P, :])

        ident = wp.tile([P, P], F16, tag="ident")
        masks.make_identity(nc, ident[:])
        c31 = wp.tile([P, 1], I32, tag="c31")
        nc.vector.memset(c31[:], 31)
        cmask = wp.tile([P, 1], I32, tag="cmask")
        nc.vector.memset(cmask[:], 0x7FFFFFFE)

        for it in range(n_tiles):
            xa = xp.tile([P, D_IN], F16, tag="xa")
            nc.sync.dma_start(xa[:], x16[it * P:(it + 1) * P, :])
            x8 = xp.tile([P, D_IN], F8, tag="x8")
            nc.sync.dma_start(x8[:], xl8[it * P:(it + 1) * P, :])
            xl = xp.tile([P, D_IN], F16, tag="xl")
            nc.scalar.activation(xl[:], x8[:], AF.Copy, scale=1.0 / XL_SCALE)

            # xT chunks [128 k-part, 128 rows] via PE transpose (hi then lo)
            xT = xp.tile([P, 2 * KC * P], F16, tag="xT")
            for j, src in ((0, xa), (1, xl)):
                for k in range(KC):
                    tps = tp.tile([P, P], F16, tag="tps")
                    nc.tensor.transpose(tps[:], src[:, k * P:(k + 1) * P],
                                        ident[:])
                    nc.scalar.copy(
                        xT[:, (j * KC + k) * P:(j * KC + k + 1) * P], tps[:])

            hs = hp.tile([P, D_OUT], F32, tag="hs")
            sparts = sp.tile([P, NBANK], F32, tag="sparts")
            n_mm = 3 * KC
            for b in range(NBANK):
                ps = pp.tile([P, 512], F32, tag="ps")
                i = 0
                for k in range(KC):
                    xh_k = xT[:, k * P:(k + 1) * P]
                    xl_k = xT[:, (KC + k) * P:(KC + k + 1) * P]
                    ws = slice(k * D_OUT + b * 512, k * D_OUT + (b + 1) * 512)
                    for lhs, rhs in ((xh_k, wh), (xh_k, wl), (xl_k, wh)):
                        nc.tensor.matmul(ps[:], lhs, rhs[:, ws],
                                         start=(i == 0), stop=(i == n_mm - 1))
                        i += 1
                nc.scalar.activation(hs[:, b * 512:(b + 1) * 512], ps[:],
                                     AF.Copy, accum_out=sparts[:, b:b + 1])

            if stage <= 1:
                nc.sync.dma_start(dbg[it * P:(it + 1) * P, :], hs[:])
                continue

            ssum = sp.tile([P, 1], F32, tag="ssum")
            nc.vector.reduce_sum(ssum[:], sparts[:], axis=AX.X)
            negmu = sp.tile([P, 1], F32, tag="negmu")
            nc.vector.tensor_scalar(out=negmu[:], in0=ssum[:],
                                    scalar1=-1.0 / D_OUT, scalar2=None,
                                    op0=AluOpType.mult)
            # hm = h - mu  (in place)
            nc.scalar.activation(hs[:], hs[:], AF.Identity, bias=negmu[:],
                                 scale=1.0)

            # pack: (bits(hm) & 0x7FFFFFFE) | sign  -> |hm| with sign in bit 0
            pk = kp.tile([P, D_OUT], I32, tag="pk")
            u = hs[:].bitcast(I32)
            nc.vector.tensor_scalar(out=pk[:], in0=u, scalar1=c31[:],
                                    scalar2=None,
                                    op0=AluOpType.logical_shift_right)
            nc.vector.scalar_tensor_tensor(out=pk[:], in0=u, scalar=cmask[:],
                                           in1=pk[:],
                                           op0=AluOpType.bitwise_and,
                                           op1=AluOpType.bitwise_or)
            pf = pk[:].bitcast(F32)

            if stage <= 2:
                nc.sync.dma_start(dbg[it * P:(it + 1) * P, :], pf)
                continue

            # L1: per-segment top-8 candidates (packed words)
            cand = cp.tile([P, NSEG * 8], F32, tag="cand")
            for s in range(NSEG):
                nc.vector.max(cand[:, s * 8:(s + 1) * 8],
                              pf[:, s * SEG:(s + 1) * SEG])

            # L2: 8 rounds of max8 + match_replace -> top-64 packed words
            vals = cp.tile([P, K], F32, tag="vals")
            mr = cp.tile([P, NSEG * 8], F32, tag="mr")
            cur = cand
            for r in range(K // 8):
                nc.vector.max(vals[:, r * 8:(r + 1) * 8], cur[:])
                if r < K // 8 - 1:
                    nxt = mr if cur is cand else cand
                    nc.vector.match_replace(nxt[:], vals[:, r * 8:(r + 1) * 8],
                                            cur[:], NEG)
                    cur = nxt

            # recover global column indices of the selected packed words
            vidx = cp.tile([P, K], U16, tag="vidx")
            for r in range(K // 8):
                nc.vector.max_index(vidx[:, r * 8:(r + 1) * 8],
                                    vals[:, r * 8:(r + 1) * 8], pf)

            nc.sync.dma_start(vout[it * P:(it + 1) * P, :], vals[:])
            nc.sync.dma_start(iout[it * P:(it + 1) * P, :], vidx[:])

    nc.compile()
    return nc


def _get_nc():
    if "nc" not in _CACHE:
        _CACHE["nc"] = _build()
    return _CACHE["nc"]


def _get_runner(nc):
    """Cached jit of the SPMD execute (mirrors bass2jax.run_bass_via_pjrt,
    but reused across calls so repeat calls skip retrace/recompile)."""
    if "runner" in _CACHE:
        return _CACHE["runner"]
    import jax
    from jax.experimental.shard_map import shard_map
    from jax.sharding import Mesh, PartitionSpec
    from concourse import bass2jax as b2j

    b2j.install_neuronx_cc_hook()

    partition_name = (nc.partition_id_tensor.name
                      if nc.partition_id_tensor else None)
    in_names, out_names, out_avals, zero_outs = [], [], [], []
    for alloc in nc.m.functions[0].allocations:
        if not isinstance(alloc, mybir.MemoryLocationSet):
            continue
        name = alloc.memorylocations[0].name
        if alloc.kind == "ExternalInput":
            if name != partition_name:
                in_names.append(name)
        elif alloc.kind == "ExternalOutput":
            shape = tuple(alloc.tensor_shape)
            dtype = mybir.dt.np(alloc.dtype)
            out_names.append(name)
            out_avals.append(jax.core.ShapedArray(shape, dtype))
            zero_outs.append(np.zeros((N_CORES * shape[0], *shape[1:]), dtype))
    n_params = len(in_names)
    n_outs = len(out_avals)
    all_names = list(in_names) + list(out_names)
    if partition_name is not None:
        all_names.append(partition_name)
    donate = tuple(range(n_params, n_params + n_outs))

    def _body(*args):
        operands = list(args)
        if partition_name is not None:
            operands.append(b2j.partition_id_tensor())
        outs = b2j._bass_exec_p.bind(
            *operands,
            out_avals=tuple(out_avals),
            in_names=tuple(all_names),
            out_names=tuple(out_names),
            lowering_input_output_aliases=(),
            sim_require_finite=True,
            sim_require_nnan=True,
            nc=nc,
        )
        return tuple(outs)

    mesh = Mesh(np.asarray(jax.devices()[:N_CORES]), ("core",))
    specs = (PartitionSpec("core"),)
    sharded = jax.jit(
        shard_map(_body, mesh=mesh,
                  in_specs=specs * (n_params + n_outs),
                  out_specs=specs * n_outs,
                  check_rep=False),
        donate_argnums=donate, keep_unused=True)
    runner = (sharded, in_names, out_names, out_avals, zero_outs)
    _CACHE["runner"] = runner
    return runner


def _run_spmd(nc, concat_map):
    """concat_map: input name -> concatenated [8*per_core, ...] array.
    Returns dict {out_name: np.ndarray of [8*per_core, ...]}."""
    try:
        sharded, in_names, out_names, out_avals, zero_outs = _get_runner(nc)
        out_arrs = sharded(*[concat_map[n] for n in in_names], *zero_outs)
        return {name: np.asarray(out_arrs[i])
                for i, name in enumerate(out_names)}
    except Exception:
        _CACHE.pop("runner", None)
        in_maps = [
            {name: arr[c * (arr.shape[0] // N_CORES):
                       (c + 1) * (arr.shape[0] // N_CORES)]
             for name, arr in concat_map.items()}
            for c in range(N_CORES)
        ]
        res = bass_utils.run_bass_kernel_spmd(
            nc, in_maps, core_ids=list(range(N_CORES)))
        return {name: np.concatenate([res.results[c][name]
                                      for c in range(N_CORES)], axis=0)
                for name in res.results[0]}


def _get_bufs():
    """Preallocated, page-faulted host buffers reused across calls (fresh
    mmaps fault slowly on this 1-cpu box once jax's threads are running)."""
    if "bufs" in _CACHE:
        return _CACHE["bufs"]
    bf = {
        "cx16": np.empty((B, D_IN), np.float16),
        "cxl8": np.empty((B, D_IN), ml_dtypes.float8_e4m3),
        "tmpx": np.empty((B, D_IN), np.float32),
        "cwh": np.empty((D_IN, D_OUT), np.float16),
        "cwl": np.empty((D_IN, D_OUT), np.float16),
        "tmpw": np.empty((D_IN, D_OUT), np.float32),
        "cwhl": np.empty((N_CORES * 2 * D_IN, WSL), np.float16),
        "dense": [np.empty((B, D_OUT), np.float32) for _ in range(2)],
    }
    for a in bf.values():
        for arr in (a if isinstance(a, list) else [a]):
            arr.fill(0)  # fault every page now, while the process is quiet
    bf["ncall"] = 0
    _CACHE["bufs"] = bf
    return bf


def _numpy_fallback(x, W, b, gamma, beta):
    h = x.astype(np.float32) @ W.astype(np.float32) + b
    mu = h.mean(-1, keepdims=True)
    var = np.square(h - mu).mean(-1, keepdims=True)
    p = (h - mu) / np.sqrt(var + 1e-5) * gamma + beta
    idx = np.argsort(-np.abs(p), axis=-1, kind="stable")[:, :K]
    sparse = np.zeros_like(p)
    np.put_along_axis(sparse, idx, np.take_along_axis(p, idx, -1), -1)
    nrm = np.linalg.norm(sparse, axis=-1, keepdims=True)
    return sparse / np.maximum(nrm, 1e-12)


def _repair_rows(dense, rows, x, W):
    """Exact fp32 recompute of the given rows (b=0/beta=0/gamma=1 math)."""
    if len(rows) == 0:
        return
    h = x[rows].astype(np.float32) @ W.astype(np.float32)
    hm = h - h.mean(-1, keepdims=True, dtype=np.float32)
    idx = np.argsort(-np.abs(hm), axis=-1, kind="stable")[:, :K]
    vals = np.take_along_axis(hm, idx, -1)
    nrm = np.maximum(np.linalg.norm(vals, axis=-1, keepdims=True), 1e-12)
    block = np.zeros((len(rows), D_OUT), np.float32)
    np.put_along_axis(block, idx, vals / nrm, -1)
    dense[rows] = block


def kernel(**inputs):
    import os, time
    prof = os.environ.get("KERNEL_PROF") == "1"
    tt = time.time
    t0 = tt()
    x = np.asarray(inputs["x"], dtype=np.float32)
    W = np.asarray(inputs["W"], dtype=np.float32)
    b = np.asarray(inputs["b"], dtype=np.float32)
    gamma = np.asarray(inputs["gamma"], dtype=np.float32)
    beta = np.asarray(inputs["beta"], dtype=np.float32)

    # kernel math relies on b == 0, beta == 0, gamma == const > 0 (per spec)
    if (np.any(b != 0) or np.any(beta != 0)
            or np.any(gamma != gamma[0]) or gamma[0] <= 0):
        return _numpy_fallback(x, W, b, gamma, beta)

    bf = _get_bufs()
    np.copyto(bf["cx16"], x, casting="unsafe")
    np.subtract(x, bf["cx16"], out=bf["tmpx"])
    np.multiply(bf["tmpx"], XL_SCALE, out=bf["tmpx"])
    try:
        np.copyto(bf["cxl8"], bf["tmpx"], casting="unsafe")
    except TypeError:
        bf["cxl8"][:] = bf["tmpx"].astype(ml_dtypes.float8_e4m3)
    np.copyto(bf["cwh"], W, casting="unsafe")
    np.subtract(W, bf["cwh"], out=bf["tmpw"])
    np.copyto(bf["cwl"], bf["tmpw"], casting="unsafe")
    for c in range(N_CORES):
        base = c * 2 * D_IN
        bf["cwhl"][base:base + D_IN] = bf["cwh"][:, c * WSL:(c + 1) * WSL]
        bf["cwhl"][base + D_IN:base + 2 * D_IN] = \
            bf["cwl"][:, c * WSL:(c + 1) * WSL]
    t1 = tt()
    nc = _get_nc()
    t2 = tt()
    res = _run_spmd(nc, {"x16": bf["cx16"], "xl8": bf["cxl8"],
                         "whl": bf["cwhl"]})
    t3 = tt()
    vals, vidx = res["vout"], res["iout"]

    # decode: sign in bit 0, magnitude in the remaining bits
    bits = np.ascontiguousarray(vals).view(np.int32)        # [B, K]
    sgn = (bits & 1).astype(np.float32)
    mag = (bits & np.int32(-2)).view(np.float32)
    val = mag * (1.0 - 2.0 * sgn)
    idx = vidx.astype(np.intp)
    nrm = np.sqrt(np.sum(np.square(mag, dtype=np.float64), axis=1,
                         keepdims=True))
    nrm = np.maximum(nrm, 1e-12)
    dense = bf["dense"][bf["ncall"] % 2]
    bf["ncall"] += 1
    dense.fill(0)
    np.put_along_axis(dense, idx, (val / nrm).astype(np.float32), axis=-1)

    # rows where a duplicated |h-mu| made max_index return the same column
    # twice (or an index escaped [0, D_OUT)): recompute exactly on host
    srt = np.sort(idx, axis=1)
    bad = (srt[:, 1:] == srt[:, :-1]).any(axis=1) | (idx >= D_OUT).any(axis=1)
    _repair_rows(dense, np.flatnonzero(bad), x, W)
    if prof:
        t4 = tt()
        print(f"[prof] prep:{t1 - t0:.2f} nc:{t2 - t1:.2f} "
              f"run:{t3 - t2:.2f} decode:{t4 - t3:.2f} "
              f"bad_rows:{int(bad.sum())}")
    return dense
